# revision 9
# baseline (speedup 1.0000x reference)
"""Distributed brute-force KNN retrieval on 8 Trainium2 NeuronCores.

queries [256, 64] f32, candidates [1M, 64] f32, ids [1M] i32
-> (top_scores [256, 100] f32, top_ids [256, 100] i32)  (sorted descending)

v2.2 design ("fp8 export, two drain routes"):
  - Shard candidates across 8 cores along N (125k each, zero-padded to
    126976 = 124 blocks x 1024 candidates).
  - Host pre-transposes candidate shards to the packed [128, N_pad/2] bf16
    layout (even 512-chunks on partitions 0:64, odd on 64:128). Per block b
    and query-group qg, two K=64 matmuls fill a psum tile [128q, 1024c] f32:
    cols 0:512 = candidates [1024b, 1024b+512) (PE row-group (0,0)),
    cols 512:1024 = [1024b+512, 1024b+1024) (row-group (64,0)).
  - PSUM is drained at the hardware limit of 1 f32 element/cycle/lane on
    each of the only two engines with PSUM read ports (walrus rejects
    tensor_tensor with two PSUM operands, and GpSimd/DMA have no PSUM port):
      V-route: one VectorE tensor_reduce max over the strided pair view
               [p, c, 2] -> fp8e4m3 [128, 512] pair maxima.
      A-route: one ScalarE activation copy -> fp8e4m3 [128, 1024] raw
               scores (the pairing happens on the host for free).
    The block route pattern (58 V : 66 A) balances the engines
    (DVE ~1.16 ns/elem vs ACT ~0.97 ns/elem).
  - Outputs per core: gpair [256, 58*512] fp8 (V-blocks), graw
    [256, 66*1024] fp8 (A-blocks).  ~25 MB out + 16.25 MB in DMA.
  - Host: reassemble a unified pair-max array [256, 63488] per core
    (max(fp8(a),fp8(b)) == fp8(max(a,b)) since the cast is monotonic),
    take top-T pairs per query (T=1024; measured worst-case needed rank on
    this input is 335), exact-rescore the union with an fp32 jax-CPU
    matmul, then top-100 with the reference's tie order (-score, id).
"""

import numpy as np
import ml_dtypes

import concourse.bass as bass
import concourse.bacc as bacc
import concourse.mybir as mybir
from concourse.tile import TileContext
from concourse.bass_utils import run_bass_kernel_spmd

B = 256            # queries
D = 64             # embedding dim
K = 100            # final top-k
N = 1_000_000      # candidates
NCORES = 8
N_PER = N // NCORES        # 125000 candidates per core
BLK = 1024                 # candidates per block / psum tile
N_BLKS = 124               # blocks per core
N_PAD = BLK * N_BLKS       # 126976 padded candidates per core
T_PAIRS = 1024             # pairs kept per query on host for exact rescore
CONTIG = False             # pair adjacent candidates (2c,2c+1) instead of (c,c+512)

# Route pattern: V = vector pair-reduce, A = scalar raw copy.
# 58 V : 66 A balances DVE (58*2*1192ns) against ACT (66*2*1038ns).
BLOCK_ROUTE = ("VAVAVAVAVAVAVAA" * 9)[:N_BLKS]
N_V = BLOCK_ROUTE.count("V")   # 58
N_A = BLOCK_ROUTE.count("A")   # 66
# output column offset of each block within gpair (V) or graw (A)
_BLOCK_OFF = np.zeros(N_BLKS, dtype=np.int64)
_vo = _ao = 0
for _b, _r in enumerate(BLOCK_ROUTE):
    if _r == "V":
        _BLOCK_OFF[_b] = _vo
        _vo += 512
    else:
        _BLOCK_OFF[_b] = _ao
        _ao += 1024
GP_COLS = _vo              # 28672
GR_COLS = _ao              # 69632

BF16 = mybir.dt.bfloat16
F32 = mybir.dt.float32
FP8 = mybir.dt.float8e4


def build_bass(
    n_blocks: int = N_BLKS,
    repeat: int = 1,
    route_override: str | None = None,
    contig_pairs: bool | None = None,
) -> bass.Bass:
    """One core's program; see module docstring."""
    route = (route_override or BLOCK_ROUTE)[:n_blocks]
    if contig_pairs is None:
        contig_pairs = CONTIG
    nv, na = route.count("V"), route.count("A")
    off = {}
    vo = ao = 0
    for b, rr in enumerate(route):
        off[b] = vo if rr == "V" else ao
        if rr == "V":
            vo += 512
        else:
            ao += 1024
    nv512 = max(vo, 512)
    na1024 = max(ao, 1024)
    nc = bacc.Bacc()
    qt = nc.dram_tensor("qt", [128, B], BF16, kind="ExternalInput")
    ct = nc.dram_tensor("ct", [128, n_blocks * 512], BF16, kind="ExternalInput")
    gpair = nc.dram_tensor("gpair", [B, nv512], FP8, kind="ExternalOutput")
    graw = nc.dram_tensor("graw", [B, na1024], FP8, kind="ExternalOutput")

    # supers: consecutive runs of 11 (then 3) blocks sharing one ctile DMA
    supers = []
    b0 = 0
    while b0 < n_blocks:
        supers.append(list(range(b0, min(b0 + 11, n_blocks))))
        b0 += 11

    with TileContext(nc) as tc:
        with (
            tc.tile_pool(name="qpool", bufs=1) as qpool,
            tc.tile_pool(name="cpool", bufs=2) as cpool,
            tc.tile_pool(name="pv", bufs=2, space="PSUM") as pv,
            tc.tile_pool(name="pa", bufs=2, space="PSUM") as pa,
            tc.tile_pool(name="ov", bufs=2) as ovp,
            tc.tile_pool(name="oa", bufs=2) as oap,
        ):
            qtile = qpool.tile([128, B], BF16)
            nc.sync.dma_start(out=qtile, in_=qt[:, :])

            def body():
                for blocks in supers:
                    w = len(blocks)
                    nvs = sum(1 for b in blocks if route[b] == "V")
                    nas = w - nvs
                    ctile = cpool.tile([128, w * 512], BF16, tag="ctile")
                    nc.sync.dma_start(
                        out=ctile,
                        in_=ct[:, blocks[0] * 512 : (blocks[0] + w) * 512],
                    )
                    omv0 = ovp.tile([128, nvs * 512], FP8, tag="omv0")
                    omv1 = ovp.tile([128, nvs * 512], FP8, tag="omv1")
                    oma0 = oap.tile([128, nas * 1024], FP8, tag="oma0")
                    oma1 = oap.tile([128, nas * 1024], FP8, tag="oma1")
                    omv = [omv0, omv1]
                    oma = [oma0, oma1]
                    vi = ai = 0
                    for j, b in enumerate(blocks):
                        cols = slice(j * 512, (j + 1) * 512)
                        is_v = route[b] == "V"
                        for qg in range(2):
                            ps = (pv if is_v else pa).tile([128, BLK], F32)
                            for h in range(2):
                                nc.tensor.matmul(
                                    ps[:, h * 512 : (h + 1) * 512],
                                    qtile[
                                        h * 64 : (h + 1) * 64,
                                        qg * 128 : (qg + 1) * 128,
                                    ],
                                    ctile[h * 64 : (h + 1) * 64, cols],
                                    start=True,
                                    stop=True,
                                )
                            if is_v:
                                nc.vector.tensor_reduce(
                                    out=omv[qg][:, vi * 512 : (vi + 1) * 512],
                                    in_=(
                                        ps.rearrange("p (g e) -> p g e", e=2)
                                        if contig_pairs
                                        else ps.rearrange("p (e g) -> p g e", e=2)
                                    ),
                                    axis=mybir.AxisListType.X,
                                    op=mybir.AluOpType.max,
                                )
                            else:
                                nc.scalar.activation(
                                    out=oma[qg][:, ai * 1024 : (ai + 1) * 1024],
                                    in_=ps,
                                    func=mybir.ActivationFunctionType.Copy,
                                )
                        if is_v:
                            vi += 1
                        else:
                            ai += 1
                    vs = [b for b in blocks if route[b] == "V"]
                    as_ = [b for b in blocks if route[b] == "A"]
                    for qg in range(2):
                        if vs:
                            o = int(off[vs[0]])
                            nc.sync.dma_start(
                                out=gpair[
                                    qg * 128 : (qg + 1) * 128,
                                    o : o + nvs * 512,
                                ],
                                in_=omv[qg],
                            )
                        if as_:
                            o = int(off[as_[0]])
                            nc.sync.dma_start(
                                out=graw[
                                    qg * 128 : (qg + 1) * 128,
                                    o : o + nas * 1024,
                                ],
                                in_=oma[qg],
                            )

            if repeat == 1:
                body()
            else:
                with tc.For_i(0, repeat, 1):
                    body()
    nc.compile()
    return nc


def prep_core_ct(cand_slice_f32: np.ndarray, n_blocks: int = N_BLKS) -> np.ndarray:
    """[<=n_blocks*1024, 64] f32 -> [128, n_blocks*512] bf16 packed layout."""
    n_pad = BLK * n_blocks
    ct = np.zeros((64, n_pad), dtype=ml_dtypes.bfloat16)
    ct[:, : cand_slice_f32.shape[0]] = np.ascontiguousarray(
        cand_slice_f32.T
    ).astype(ml_dtypes.bfloat16)
    A = ct.reshape(64, n_pad // 512, 512)
    return np.ascontiguousarray(
        np.concatenate(
            [A[:, 0::2, :].reshape(64, -1), A[:, 1::2, :].reshape(64, -1)], axis=0
        )
    )


def prep_qt(queries_f32: np.ndarray) -> np.ndarray:
    qt = np.ascontiguousarray(queries_f32.T).astype(ml_dtypes.bfloat16)  # [64, 256]
    return np.ascontiguousarray(np.concatenate([qt, qt], axis=0))  # [128, 256]


def assemble_pairs(gpair_f32: np.ndarray, graw_f32: np.ndarray) -> np.ndarray:
    """Per-core outputs -> unified pair-max array [B, N_BLKS*512]."""
    out = np.empty((B, N_BLKS * 512), dtype=np.float32)
    for b in range(N_BLKS):
        o = int(_BLOCK_OFF[b])
        dst = out[:, b * 512 : (b + 1) * 512]
        if BLOCK_ROUTE[b] == "V":
            dst[:] = gpair_f32[:, o : o + 512]
        elif CONTIG:
            np.maximum(
                graw_f32[:, o : o + 1024 : 2],
                graw_f32[:, o + 1 : o + 1024 : 2],
                out=dst,
            )
        else:
            np.maximum(
                graw_f32[:, o : o + 512],
                graw_f32[:, o + 512 : o + 1024],
                out=dst,
            )
    return out


def host_merge(q_f32, c_f32, ids_np, pairs_f32):
    """pairs_f32: [NCORES, B, N_BLKS*512] -> exact (top_scores, top_ids)."""
    import jax
    import jax.numpy as jnp

    gcols = N_BLKS * 512
    flat = np.ascontiguousarray(pairs_f32.transpose(1, 0, 2)).reshape(
        B, NCORES * gcols
    )
    top_g = np.argpartition(-flat, T_PAIRS - 1, axis=1)[:, :T_PAIRS]  # [B, T]
    core = top_g // gcols
    r = top_g % gcols
    b = r // 512
    c = r % 512
    if CONTIG:
        local = 1024 * b + 2 * c                                # [B, T]
        offs = np.array([0, 1], dtype=np.int64)
    else:
        local = 1024 * b + c                                    # [B, T]
        offs = np.array([0, 512], dtype=np.int64)
    cand_ids = core[:, :, None] * N_PER + local[:, :, None] + offs[None, None, :]
    valid = (local[:, :, None] + offs[None, None, :]) < N_PER
    cand_ids = cand_ids.reshape(B, -1)
    valid = valid.reshape(B, -1)
    safe = np.where(valid, cand_ids, 0)
    uniq, inv = np.unique(safe, return_inverse=True)
    pad_u = -(-len(uniq) // 16384) * 16384  # stable shapes -> stable jit cache
    uniq_pad = np.zeros(pad_u, dtype=uniq.dtype)
    uniq_pad[: len(uniq)] = uniq
    cpu = jax.local_devices(backend="cpu")[0]
    with jax.default_device(cpu):
        sub = np.asarray(jnp.matmul(q_f32, c_f32[uniq_pad].T))  # [B, pad_u]
    scores = sub[np.arange(B)[:, None], inv.reshape(B, -1)]
    scores = np.where(valid, scores, -np.inf)
    top_idx = np.argpartition(-scores, K - 1, axis=1)[:, :K]
    top_sc = np.take_along_axis(scores, top_idx, axis=1)
    top_id = np.take_along_axis(safe, top_idx, axis=1)
    order = np.lexsort((top_id, -top_sc), axis=1)
    top_sc = np.take_along_axis(top_sc, order, axis=1)
    top_id = np.take_along_axis(top_id, order, axis=1)
    return (
        top_sc.astype(np.float32),
        np.asarray(ids_np)[top_id].astype(np.asarray(ids_np).dtype),
    )


_NC_CACHE: dict = {}
TRACE = False          # test harness can flip this to capture a profile
LAST_RESULTS = None    # BassKernelResults from the most recent run
_LAST_IN_MAPS = None   # per-core input dict from the most recent run


def last_in_maps():
    return _LAST_IN_MAPS


def _get_nc() -> bass.Bass:
    if "nc" not in _NC_CACHE:
        _NC_CACHE["nc"] = build_bass()
    return _NC_CACHE["nc"]


def kernel(queries, candidates, ids):
    global LAST_RESULTS, _LAST_IN_MAPS
    q = np.asarray(queries, dtype=np.float32)
    c = np.asarray(candidates, dtype=np.float32)
    ids_np = np.asarray(ids)

    qt2 = prep_qt(q)
    in_maps = []
    for core in range(NCORES):
        in_maps.append(
            {"qt": qt2, "ct": prep_core_ct(c[core * N_PER : (core + 1) * N_PER])}
        )

    _LAST_IN_MAPS = in_maps
    res = run_bass_kernel_spmd(
        _get_nc(), in_maps, core_ids=list(range(NCORES)), trace=TRACE
    )
    LAST_RESULTS = res
    pairs = np.stack(
        [
            assemble_pairs(
                np.asarray(r["gpair"]).astype(np.float32),
                np.asarray(r["graw"]).astype(np.float32),
            )
            for r in res.results
        ]
    )
    return host_merge(q, c, ids_np, pairs)
